# revision 63
# baseline (speedup 1.0000x reference)
"""Trainium2 Bass kernel for nn_CustomCrossModalAttention (B=2, N=2048, D=768, H=12).

Sharding (8 cores, ZERO collectives):
  - core c owns batch b = c//4 and query rows [512*(c%4), 512*(c%4)+512).
  - k' and v are computed REDUNDANTLY for all 2048 keys of the core's batch
    (the cost-model prices AllGather at 15us + out_bytes/40GB/s, so the two
    baseline gathers cost 267us -- far more than the +46us of replicated PE
    matmul work).

Structure (engine-balance driven):
  phase1: q proj -> k' proj (transposed copy-out with pos fold) -> v chunks
          0..11. All LN rstds here use ACT Sqrt + DVE reciprocal -- they all
          complete BEFORE the first exp so the ACT table never thrashes.
  attention: heads stream with 3-chunk score groups -> exp (ACT-bound).
          v chunks 12..15 are interleaved into heads 0..1 with a DVE-only
          Newton rsqrt (no ACT table) + ACT Identity apply (Identity lives
          in every table). attn@v is re-oriented to [queries, dims] (full
          M=128) and lags the exp stream by 2 heads (at pool bufs=3).
  phase3: out proj, gate (sigmoid = 1/(1+exp(-x)) built from the already
          loaded exp table), fuse, final LN (single Sqrt table load).

Algebra (exact): scores*scale + q@pos == scale * (q @ (k + pos/scale)^T);
LN_v gain/bias folded into wo / bo on the host; softmax denominator via a
ones column appended to v.

Hardware constraints honored: GPSIMD never touches PSUM; all matmuls and
PE transposes in bf16 (1 cyc/row); psum pools sized to the 8x2KB banks.
"""

import numpy as np
import ml_dtypes

B, N, D = 2, 2048, 768
H, DH = 12, 64
P = 128
CORES, GROUP = 8, 4
S = 512            # query rows per core
NCH = S // P       # 4 own row chunks
MCH = N // P       # 16 key row chunks
G6 = D // P        # 6
SCALE = DH ** -0.5
EPS = 1e-5

BF = ml_dtypes.bfloat16

_CACHE = {}


def _build(has_bqkv, has_bo, has_gb):
    from contextlib import ExitStack

    import concourse.bacc as bacc
    import concourse.mybir as mybir
    import concourse.tile as tile
    from concourse.masks import make_identity

    f32 = mybir.dt.float32
    bf16 = mybir.dt.bfloat16
    ALU = mybir.AluOpType
    ACTF = mybir.ActivationFunctionType

    nc = bacc.Bacc("TRN2", target_bir_lowering=False, num_devices=CORES)

    def din(name, shape, dt=bf16):
        return nc.dram_tensor(name, shape, dt, kind="ExternalInput")

    xqT = din("xqT", [D, S])            # own infrared rows, transposed
    xvTf = din("xvTf", [D, N])          # FULL batch visible rows, transposed
    visT_own = din("visT_own", [D, S])  # own visible rows, transposed (gate)
    vis_nat = din("vis_nat", [S, D], f32)
    posTb = din("posTb", [D, N])        # (pos/scale + lnk_b), transposed
    wqkvT = din("wqkvT", [D, 3 * (D + 1)])
    woT = din("woT", [D, D])            # (wo * lnv_w).T
    gwT = din("gwT", [2 * D, D])
    lnq_g = din("lnq_g", [P, G6], f32)
    lnq_b = din("lnq_b", [P, G6], f32)
    lnk_g = din("lnk_g", [P, G6], f32)
    lnf = din("lnf", [2, D])
    bqkv = din("bqkv", [1, 3 * (D + 1)]) if has_bqkv else None
    bo_a = din("bo_a", [1, D]) if has_bo else None
    gb = din("gb", [1, D]) if has_gb else None
    out_rows = nc.dram_tensor("out_rows", [S, D], f32, kind="ExternalOutput")

    VPH1 = 12        # v chunks computed in phase 1 (ACT sqrt era)

    with tile.TileContext(nc) as tc, ExitStack() as ctx:
        const = ctx.enter_context(tc.tile_pool(name="const", bufs=1))

        ident = const.tile([P, P], bf16)
        make_identity(nc, ident)
        eps_t = const.tile([P, 1], f32)
        nc.vector.memset(eps_t, EPS)
        one_t = const.tile([P, 1], f32)
        nc.vector.memset(one_t, 1.0)
        ones_bf = const.tile([1, P], bf16)
        nc.vector.memset(ones_bf, 1.0)

        lnq_g_sb = const.tile([P, G6], f32)
        nc.scalar.dma_start(out=lnq_g_sb, in_=lnq_g.ap())
        lnq_b_sb = const.tile([P, G6], f32)
        nc.scalar.dma_start(out=lnq_b_sb, in_=lnq_b.ap())
        lnk_g_sb = const.tile([P, G6], f32)
        nc.scalar.dma_start(out=lnk_g_sb, in_=lnk_g.ap())
        woT_sb = const.tile([P, G6, D], bf16)
        nc.scalar.dma_start(out=woT_sb,
                            in_=woT.rearrange("(s p) o -> p s o", p=P))

        qT_sb = const.tile([P, G6, S], bf16)
        kT_sb = const.tile([P, G6, N], bf16)
        outT_sb = const.tile([P, G6, S], bf16)
        if has_bqkv:
            bqkv_sb = const.tile([1, 3 * (D + 1)], bf16)
            nc.scalar.dma_start(out=bqkv_sb, in_=bqkv.ap())

        HALves = [(0, 512), (512, D + 1)]
        HAL768 = [(0, 512), (512, D)]

        def proj_chunk(py, lhsT_sb, w_sb, w_off, c):
            for o0, o1 in HALves:
                for s in range(G6):
                    nc.tensor.matmul(
                        py[:, o0:o1],
                        lhsT_sb[:, s, c * P:(c + 1) * P],
                        w_sb[:, s, o0:o1],
                        start=(s == 0), stop=(not has_bqkv and s == G6 - 1),
                    )
                if has_bqkv:
                    nc.tensor.matmul(
                        py[:, o0:o1], ones_bf,
                        bqkv_sb[:, w_off + o0:w_off + o1],
                        start=False, stop=True,
                    )

        def stats(py, pool):
            st = pool.tile([P, 2, 6], f32, tag="st")
            for i in range(2):
                nc.vector.bn_stats(out=st[:, i], in_=py[:, i * 384:(i + 1) * 384])
            mv = pool.tile([P, 2], f32, tag="mv")
            nc.vector.bn_aggr(out=mv, in_=st)
            return mv

        def rstd_of(mv, pool, tag="rs"):
            """rstd = 1/sqrt(var+eps): ACT Sqrt then DVE reciprocal."""
            rstd = pool.tile([P, 1], f32, tag=tag + "r")
            nc.scalar.activation(out=rstd, in_=mv[:, 1:2], func=ACTF.Sqrt,
                                 bias=eps_t, scale=1.0)
            nc.vector.reciprocal(out=rstd, in_=rstd)
            return rstd

        def negmr_of(mv, rstd, pool, tag="nm"):
            negmr = pool.tile([P, 1], f32, tag=tag)
            nc.vector.tensor_scalar(
                out=negmr, in0=mv[:, 0:1], scalar1=rstd, scalar2=-1.0,
                op0=ALU.mult, op1=ALU.mult,
            )
            return negmr

        mid = ctx.enter_context(tc.tile_pool(name="mid", bufs=1))
        vaug_sb = mid.tile([P, MCH, H, DH + 1], bf16)
        nc.vector.memset(vaug_sb[:, :, :, DH:DH + 1], 1.0)
        onat_sb = mid.tile([P, NCH, H, DH], bf16)

        rinp = ctx.enter_context(tc.tile_pool(name="rin", bufs=4))
        w2 = ctx.enter_context(tc.tile_pool(name="w2", bufs=4))

        GROUPS6 = [(15, 1), (0, 3), (3, 3), (6, 3), (9, 3), (12, 3)]

        def scores_group(ps_pool, at_h, h, mc0, w):
            p0 = DH * (h % 2)
            grp = h // 2
            ps = ps_pool.tile([P, 3, S], f32, tag="ps3")
            for j in range(w):
                mc = mc0 + j
                nc.tensor.matmul(
                    ps[:, j],
                    kT_sb[p0:p0 + DH, grp, mc * P:(mc + 1) * P],
                    qT_sb[p0:p0 + DH, grp, :],
                    start=True, stop=True,
                )
            nc.scalar.activation(
                out=at_h[:, mc0:mc0 + w, :], in_=ps[:, :w],
                func=ACTF.Exp, scale=SCALE,
            )

        def av_head(ps_o, at_h, h):
            po = ps_o.tile([P, NCH, DH + 1], f32, tag="po")
            for qc in range(NCH):
                for mc in range(MCH):
                    nc.tensor.matmul(
                        po[:, qc],
                        at_h[:, mc, qc * P:(qc + 1) * P],
                        vaug_sb[:, mc, h, :],
                        start=(mc == 0), stop=(mc == MCH - 1),
                    )
                rinv = rinp.tile([P, 1], f32, tag="rin")
                nc.vector.reciprocal(out=rinv, in_=po[:, qc, DH:DH + 1])
                nc.vector.tensor_scalar_mul(
                    out=onat_sb[:, qc, h], in0=po[:, qc, 0:DH], scalar1=rinv,
                )

        def newton_rstd(mv):
            """rsqrt(var+eps) purely on DVE: r0 = 1/(var+eps), then two
            Newton steps r <- r*(1.5 - 0.5*x*r^2). Converges fast since the
            projection row variance concentrates near 1."""
            x = w2.tile([P, 1], f32, tag="nx")
            nc.vector.tensor_scalar_add(out=x, in0=mv[:, 1:2], scalar1=EPS)
            r = w2.tile([P, 1], f32, tag="nr")
            nc.vector.reciprocal(out=r, in_=x)
            a = w2.tile([P, 1], f32, tag="na")
            for _ in range(2):
                nc.vector.tensor_tensor(out=a, in0=r, in1=r, op=ALU.mult)
                nc.vector.tensor_tensor(out=a, in0=a, in1=x, op=ALU.mult)
                nc.vector.tensor_scalar(
                    out=a, in0=a, scalar1=-0.5, scalar2=1.5,
                    op0=ALU.mult, op1=ALU.add,
                )
                nc.vector.tensor_tensor(out=r, in0=r, in1=a, op=ALU.mult)
            return r

        at_tiles = {}
        atp_cm = tc.tile_pool(name="atp", bufs=3)
        atp = atp_cm.__enter__()
        with tc.tile_pool(name="xvw", bufs=1) as xvw:
            xvT_sb = xvw.tile([P, G6, N], bf16)
            wv_sb = xvw.tile([P, G6, D + 1], bf16)

            # ---------------- phase 1: q, k', v[0:12] ----------------
            with (
                tc.tile_pool(name="ph1", bufs=1) as ph1,
                tc.tile_pool(name="knp", bufs=3) as knp,
                tc.tile_pool(name="post", bufs=2) as postp,
                tc.tile_pool(name="stat", bufs=4) as statp,
                tc.tile_pool(name="ps_p", bufs=3, space="PSUM") as ps_p,
                tc.tile_pool(name="ps_t", bufs=1, space="PSUM") as ps_t,
            ):
                xqT_sb = ph1.tile([P, G6, S], bf16)
                nc.sync.dma_start(out=xqT_sb,
                                  in_=xqT.rearrange("(s p) n -> p s n", p=P))
                nc.gpsimd.dma_start(out=xvT_sb,
                                    in_=xvTf.rearrange("(s p) n -> p s n", p=P))
                wq_sb = ph1.tile([P, G6, D + 1], bf16)
                for h0, h1 in HALves:
                    nc.sync.dma_start(
                        out=wq_sb[:, :, h0:h1],
                        in_=wqkvT.rearrange("(s p) o -> p s o", p=P)[:, :, h0:h1],
                    )
                wk_sb = ph1.tile([P, G6, D + 1], bf16)
                nc.sync.dma_start(
                    out=wk_sb,
                    in_=wqkvT.rearrange("(s p) o -> p s o", p=P)[
                        :, :, D + 1:2 * (D + 1)],
                )
                nc.sync.dma_start(
                    out=wv_sb,
                    in_=wqkvT.rearrange("(s p) o -> p s o", p=P)[
                        :, :, 2 * (D + 1):3 * (D + 1)],
                )
                qnat = ph1.tile([P, NCH, D], bf16)

                for c in range(NCH):
                    py = ps_p.tile([P, D + 1], f32, tag="py")
                    proj_chunk(py, xqT_sb, wq_sb, 0, c)
                    mv = stats(py, statp)
                    rstd = rstd_of(mv, statp, "qr")
                    negmr = negmr_of(mv, rstd, statp, "qn")
                    nc.scalar.activation(
                        out=qnat[:, c], in_=py[:, 0:D], func=ACTF.Identity,
                        bias=negmr, scale=rstd,
                    )

                scrp = ph1.tile([P, 2, D], bf16)

                # v chunks 0..11: rstd via ACT sqrt (still before any exp)
                for c in range(VPH1):
                    pv = ps_p.tile([P, D + 1], f32, tag="py")
                    proj_chunk(pv, xvT_sb, wv_sb, 2 * (D + 1), c)
                    mv = stats(pv, statp)
                    rstd = rstd_of(mv, statp, "vr")
                    negmr = negmr_of(mv, rstd, statp, "vn")
                    nc.scalar.activation(
                        out=vaug_sb[:, c, :, 0:DH],
                        in_=pv[:, 0:D].rearrange("p (h d) -> p h d", h=H),
                        func=ACTF.Identity, bias=negmr, scale=rstd,
                    )

                def kchunk(c, pend):
                    py = ps_p.tile([P, D + 1], f32, tag="py")
                    proj_chunk(py, xvT_sb, wk_sb, D + 1, c)
                    # stats on ACT: sum(y^2) via Square+accumulate; mean came
                    # free out of the matmul's appended wsum/768 column.
                    sumsq = statp.tile([P, 1], f32, tag="ksq")
                    nc.scalar.activation(
                        out=scrp[:, c % 2], in_=py[:, 0:D], func=ACTF.Square,
                        accum_out=sumsq,
                    )
                    mean = statp.tile([P, 1], f32, tag="kmn")
                    nc.vector.tensor_scalar_mul(out=mean, in0=py[:, D:D + 1],
                                                scalar1=1.0)
                    var = statp.tile([P, 1], f32, tag="kvr")
                    nc.vector.tensor_scalar(
                        out=var, in0=mean, scalar1=mean, scalar2=-1.0,
                        op0=ALU.mult, op1=ALU.mult,
                    )
                    nc.vector.tensor_scalar(
                        out=var, in0=sumsq, scalar1=1.0 / D, scalar2=var,
                        op0=ALU.mult, op1=ALU.add,
                    )
                    mv = statp.tile([P, 2], f32, tag="kmv")
                    nc.vector.tensor_copy(out=mv[:, 0:1], in_=mean)
                    nc.vector.tensor_copy(out=mv[:, 1:2], in_=var)
                    rstd = rstd_of(mv, statp, "kr")
                    negmr = negmr_of(mv, rstd, statp, "kn")
                    pair = c // 2
                    if c % 2 == 0:
                        pend["t"] = knp.tile([P, 2, D], bf16, tag="knat",
                                             name=f"knat{c}")
                    nc.scalar.activation(
                        out=pend["t"][:, c % 2], in_=py[:, 0:D],
                        func=ACTF.Identity, bias=negmr, scale=rstd,
                    )
                    pend[pair] = pend["t"]

                def ktranspose(pair, pend):
                    knat2 = pend.pop(pair)
                    pos_sb = postp.tile([P, G6, 2 * P], bf16, tag="pos")
                    nc.sync.dma_start(
                        out=pos_sb,
                        in_=posTb.rearrange("(s p) n -> p s n", p=P)[
                            :, :, pair * 2 * P:(pair + 1) * 2 * P],
                    )
                    pt = ps_t.tile([P, G6, 2 * P], bf16, tag="pt")
                    for i in range(2):
                        for s in range(G6):
                            nc.tensor.transpose(
                                pt[:, s, i * P:(i + 1) * P],
                                knat2[:, i, s * P:(s + 1) * P], ident,
                            )
                    for s in range(G6):
                        nc.vector.scalar_tensor_tensor(
                            out=kT_sb[:, s, pair * 2 * P:(pair + 1) * 2 * P],
                            in0=pt[:, s], scalar=lnk_g_sb[:, s:s + 1],
                            in1=pos_sb[:, s], op0=ALU.mult, op1=ALU.add,
                        )

                pend = {}
                for c in range(MCH):
                    kchunk(c, pend)
                    if c == 1:
                        for g in range(2):
                            ptq = ps_t.tile([P, G6, 2 * P], bf16, tag="pt")
                            for i in range(2):
                                qc = g * 2 + i
                                for s in range(G6):
                                    nc.tensor.transpose(
                                        ptq[:, s, i * P:(i + 1) * P],
                                        qnat[:, qc, s * P:(s + 1) * P], ident,
                                    )
                            for s in range(G6):
                                nc.vector.scalar_tensor_tensor(
                                    out=qT_sb[:, s, g * 2 * P:(g + 1) * 2 * P],
                                    in0=ptq[:, s], scalar=lnq_g_sb[:, s:s + 1],
                                    in1=lnq_b_sb[:, s:s + 1].to_broadcast(
                                        [P, 2 * P]),
                                    op0=ALU.mult, op1=ALU.add,
                                )
                    if c >= 3 and c % 2 == 1:
                        ktranspose((c - 1) // 2 - 1, pend)
                ktranspose(MCH // 2 - 1, pend)

            # ---------------- attention part A: heads 0..1 + v[12:16] ------
            with (
                tc.tile_pool(name="ps_sA", bufs=2, space="PSUM") as ps_sA,
                tc.tile_pool(name="ps_v", bufs=1, space="PSUM") as ps_v,
            ):
                vq = list(range(VPH1, MCH))
                for h in range(2):
                    at_h = atp.tile([P, MCH, S], bf16, tag="at", name=f"at{h}")
                    at_tiles[h] = at_h
                    for gi, (mc0, w) in enumerate(GROUPS6):
                        if gi in (0, 3) and vq:
                            c = vq.pop(0)
                            pv = ps_v.tile([P, D + 1], f32, tag="pv")
                            proj_chunk(pv, xvT_sb, wv_sb, 2 * (D + 1), c)
                            mv = stats(pv, w2)
                            r = newton_rstd(mv)
                            negmr = negmr_of(mv, r, w2, "wn")
                            nc.scalar.activation(
                                out=vaug_sb[:, c, :, 0:DH],
                                in_=pv[:, 0:D].rearrange("p (h d) -> p h d", h=H),
                                func=ACTF.Identity, bias=negmr, scale=r,
                            )
                        scores_group(ps_sA, at_h, h, mc0, w)

        # ---------------- attention part B: heads 2..11 + lagged av --------
        with (
            tc.tile_pool(name="ps_o", bufs=2, space="PSUM") as ps_o,
            tc.tile_pool(name="ps_sB", bufs=2, space="PSUM") as ps_sB,
        ):
            for h in range(2, H):
                at_h = atp.tile([P, MCH, S], bf16, tag="at", name=f"at{h}")
                at_tiles[h] = at_h
                if h == H - 1:
                    # last head: keep its score stream dense so the final
                    # exps (and the whole tail) start as early as possible
                    av_head(ps_o, at_tiles.pop(h - 2), h - 2)
                for gi, (mc0, w) in enumerate(GROUPS6):
                    scores_group(ps_sB, at_h, h, mc0, w)
                    if gi == 0 and h < H - 1:
                        # av for h-2 after this head's first exp is fed
                        av_head(ps_o, at_tiles.pop(h - 2), h - 2)
            av_head(ps_o, at_tiles.pop(H - 2), H - 2)
            av_head(ps_o, at_tiles.pop(H - 1), H - 1)
        atp_cm.__exit__(None, None, None)

        # transpose attention output for the output projection
        with tc.tile_pool(name="ps_t2", bufs=1, space="PSUM") as ps_t2:
            for g in range(2):
                pt = ps_t2.tile([P, G6, 2 * P], bf16, tag="pt2")
                for i in range(2):
                    qc = g * 2 + i
                    src = onat_sb[:, qc].rearrange("p h d -> p (h d)")
                    for s in range(G6):
                        nc.tensor.transpose(
                            pt[:, s, i * P:(i + 1) * P],
                            src[:, s * P:(s + 1) * P], ident,
                        )
                if g == 0:
                    nc.vector.tensor_copy(
                        out=outT_sb[:, :, g * 2 * P:(g + 1) * 2 * P],
                        in_=pt,
                    )
                else:
                    nc.scalar.copy(
                        out=outT_sb[:, :, g * 2 * P:(g + 1) * 2 * P],
                        in_=pt,
                    )

        # ---------------- phase 3: out proj, gate, fuse, final LN ----------
        with (
            tc.tile_pool(name="ph3", bufs=1) as ph3,
            tc.tile_pool(name="fw", bufs=2) as fw,
            tc.tile_pool(name="st3", bufs=4) as st3,
            tc.tile_pool(name="ps_z", bufs=2, space="PSUM") as ps_z,
            tc.tile_pool(name="ps_t3", bufs=1, space="PSUM") as ps_t3,
        ):
            ident_f32 = ph3.tile([P, P], f32)
            make_identity(nc, ident_f32)
            lnfw_sb = ph3.tile([1, D], bf16)
            nc.scalar.dma_start(out=lnfw_sb, in_=lnf.ap()[0:1, :])
            lnfb_sb = ph3.tile([1, D], bf16)
            nc.scalar.dma_start(out=lnfb_sb, in_=lnf.ap()[1:2, :])
            vis_sb = ph3.tile([P, NCH, D], f32)
            nc.scalar.dma_start(out=vis_sb,
                                in_=vis_nat.rearrange("(c p) o -> p c o", p=P))
            visT_sb = ph3.tile([P, G6, S], bf16)
            nc.scalar.dma_start(out=visT_sb,
                                in_=visT_own.rearrange("(s p) n -> p s n", p=P))
            gwv_sb = ph3.tile([P, G6, D], bf16)
            nc.sync.dma_start(
                out=gwv_sb,
                in_=gwT.rearrange("(s p) o -> p s o", p=P)[0:P, 0:G6, :],
            )
            gwz_sb = ph3.tile([P, G6, D], bf16)
            nc.sync.dma_start(
                out=gwz_sb,
                in_=gwT.rearrange("(g s p) o -> p (g s) o", p=P, g=2)[:, G6:, :],
            )
            if has_gb:
                gb_sb = ph3.tile([1, D], bf16)
                nc.sync.dma_start(out=gb_sb, in_=gb.ap())
            if has_bo:
                bo_sb = ph3.tile([1, D], bf16)
                nc.sync.dma_start(out=bo_sb, in_=bo_a.ap())

            z_sb = ph3.tile([P, NCH, D], f32)
            zT_sb = ph3.tile([P, G6, S], bf16)
            gsig = ph3.tile([P, NCH, D], f32)
            gbc = ph3.tile([P, D], f32)
            bbc = ph3.tile([P, D], f32)

            # broadcast final-LN gain/bias (cheap bf16; also warms PE)
            for dst, src_row in ((gbc, lnfw_sb), (bbc, lnfb_sb)):
                pb = ps_z.tile([P, D], f32, tag="pz")
                for o0, o1 in HAL768:
                    nc.tensor.matmul(
                        pb[:, o0:o1], ones_bf, src_row[:, o0:o1],
                        start=True, stop=True,
                    )
                nc.vector.tensor_copy(out=dst, in_=pb)

            # z = attout @ woT' (+bo)
            for c in range(NCH):
                pz = ps_z.tile([P, D], f32, tag="pz")
                for o0, o1 in HAL768:
                    for s in range(G6):
                        nc.tensor.matmul(
                            pz[:, o0:o1],
                            outT_sb[:, s, c * P:(c + 1) * P],
                            woT_sb[:, s, o0:o1],
                            start=(s == 0), stop=(not has_bo and s == G6 - 1),
                        )
                    if has_bo:
                        nc.tensor.matmul(
                            pz[:, o0:o1], ones_bf, bo_sb[:, o0:o1],
                            start=False, stop=True,
                        )
                nc.scalar.copy(out=z_sb[:, c], in_=pz)

            # zT for the gate matmul
            for g in range(2):
                pt = ps_t3.tile([P, G6, 2 * P], f32, tag="pt3")
                for i in range(2):
                    c = g * 2 + i
                    for s in range(G6):
                        nc.tensor.transpose(
                            pt[:, s, i * P:(i + 1) * P],
                            z_sb[:, c, s * P:(s + 1) * P], ident_f32,
                        )
                if g == 0:
                    nc.vector.tensor_copy(
                        out=zT_sb[:, :, g * 2 * P:(g + 1) * 2 * P], in_=pt,
                    )
                else:
                    nc.scalar.copy(
                        out=zT_sb[:, :, g * 2 * P:(g + 1) * 2 * P], in_=pt,
                    )

            # gate: sigmoid(x) = 1/(1+exp(-x)) -- exp stays in the exp table
            for c in range(NCH):
                pg = ps_z.tile([P, D], f32, tag="pz")
                for o0, o1 in HAL768:
                    for s in range(G6):
                        nc.tensor.matmul(
                            pg[:, o0:o1],
                            visT_sb[:, s, c * P:(c + 1) * P],
                            gwv_sb[:, s, o0:o1],
                            start=(s == 0), stop=False,
                        )
                    for s in range(G6):
                        nc.tensor.matmul(
                            pg[:, o0:o1],
                            zT_sb[:, s, c * P:(c + 1) * P],
                            gwz_sb[:, s, o0:o1],
                            start=False,
                            stop=(not has_gb and s == G6 - 1),
                        )
                    if has_gb:
                        nc.tensor.matmul(
                            pg[:, o0:o1], ones_bf, gb_sb[:, o0:o1],
                            start=False, stop=True,
                        )
                ge = fw.tile([P, D], f32, tag="ge", name=f"ge{c}")
                nc.scalar.activation(out=ge, in_=pg, func=ACTF.Exp, scale=-1.0)
                nc.scalar.activation(out=ge, in_=ge, func=ACTF.Identity,
                                     bias=one_t, scale=1.0)
                nc.vector.reciprocal(out=gsig[:, c], in_=ge)

            # fuse + final LN (stats first, one batched sqrt table load)
            fmv = ph3.tile([P, NCH, 2], f32)
            fus4 = gsig  # gsig is dead after the fuse multiply; reuse in place
            for c in range(NCH):
                dvz = fw.tile([P, D], f32, tag="dvz")
                nc.gpsimd.tensor_tensor(
                    out=dvz, in0=vis_sb[:, c], in1=z_sb[:, c], op=ALU.subtract,
                )
                nc.vector.tensor_tensor(out=fus4[:, c], in0=gsig[:, c],
                                        in1=dvz, op=ALU.mult)
                nc.vector.tensor_tensor(out=fus4[:, c], in0=fus4[:, c],
                                        in1=z_sb[:, c], op=ALU.add)
                st = st3.tile([P, 2, 6], f32, tag="st")
                for i in range(2):
                    nc.vector.bn_stats(out=st[:, i],
                                       in_=fus4[:, c, i * 384:(i + 1) * 384])
                nc.vector.bn_aggr(out=fmv[:, c], in_=st)
            frstd = ph3.tile([P, NCH], f32)
            nc.scalar.activation(out=frstd, in_=fmv[:, :, 1], func=ACTF.Sqrt,
                                 bias=eps_t, scale=1.0)
            nc.vector.reciprocal(out=frstd, in_=frstd)
            fnegmr = ph3.tile([P, NCH], f32)
            nc.vector.tensor_tensor(out=fnegmr, in0=fmv[:, :, 0], in1=frstd,
                                    op=ALU.mult)
            nc.vector.tensor_scalar_mul(out=fnegmr, in0=fnegmr, scalar1=-1.0)
            for c in range(NCH):
                tnorm = fw.tile([P, D], f32, tag="tn")
                nc.scalar.activation(
                    out=tnorm, in_=fus4[:, c], func=ACTF.Identity,
                    bias=fnegmr[:, c:c + 1], scale=frstd[:, c:c + 1],
                )
                nc.vector.tensor_tensor(out=tnorm, in0=tnorm, in1=gbc,
                                        op=ALU.mult)
                nc.vector.tensor_tensor(out=tnorm, in0=tnorm, in1=bbc,
                                        op=ALU.add)
                nc.sync.dma_start(
                    out=out_rows.rearrange("(c p) o -> p c o", p=P)[:, c],
                    in_=tnorm,
                )

    nc.compile()
    return nc


def _prepare_in_maps(inputs):
    f32 = np.float32
    vis = np.asarray(inputs["visible_features"], f32)
    inf = np.asarray(inputs["infrared_features"], f32)
    wq = np.asarray(inputs["wq"], f32)
    bq = np.asarray(inputs["bq"], f32)
    lnq_w = np.asarray(inputs["lnq_w"], f32)
    lnq_b = np.asarray(inputs["lnq_b"], f32)
    wk = np.asarray(inputs["wk"], f32)
    bk = np.asarray(inputs["bk"], f32)
    lnk_w = np.asarray(inputs["lnk_w"], f32)
    lnk_b = np.asarray(inputs["lnk_b"], f32)
    wv = np.asarray(inputs["wv"], f32)
    bv = np.asarray(inputs["bv"], f32)
    lnv_w = np.asarray(inputs["lnv_w"], f32)
    lnv_b = np.asarray(inputs["lnv_b"], f32)
    pos = np.asarray(inputs["pos_emb"], f32)[:N]
    wo = np.asarray(inputs["wo"], f32)
    bo = np.asarray(inputs["bo"], f32)
    gw = np.asarray(inputs["gate_w"], f32)
    gb_ = np.asarray(inputs["gate_b"], f32)
    ln_w = np.asarray(inputs["ln_w"], f32)
    ln_b = np.asarray(inputs["ln_b"], f32)

    blocks = []
    for w in (wq, wk, wv):
        wt = w.T.astype(np.float32)
        blocks.append(np.concatenate([wt, wt.mean(axis=1, keepdims=True)],
                                     axis=1))
    wqkvT = np.concatenate(blocks, axis=1).astype(BF)
    bqkv = np.concatenate(
        [np.concatenate([b, [b.mean()]]) for b in (bq, bk, bv)])[None]
    woT = ((wo * lnv_w[None, :]).T).astype(BF)
    bo_a = (bo + wo @ lnv_b)[None]
    gwT = gw.T.astype(BF)
    lnq_g = np.ascontiguousarray(lnq_w.reshape(G6, P).T)
    lnq_b2 = np.ascontiguousarray(lnq_b.reshape(G6, P).T)
    lnk_g = np.ascontiguousarray(lnk_w.reshape(G6, P).T)
    lnf = np.stack([ln_w, ln_b]).astype(BF)
    flags = (
        bool(np.any(bqkv)), bool(np.any(bo_a)), bool(np.any(gb_)),
    )

    in_maps = []
    for c in range(CORES):
        b, r0 = c // GROUP, (c % GROUP) * S
        m = {
            "xqT": np.ascontiguousarray(inf[b, r0:r0 + S].T).astype(BF),
            "xvTf": np.ascontiguousarray(vis[b].T).astype(BF),
            "visT_own": np.ascontiguousarray(vis[b, r0:r0 + S].T).astype(BF),
            "vis_nat": np.ascontiguousarray(vis[b, r0:r0 + S]),
            "posTb": np.ascontiguousarray(
                pos.T / SCALE + lnk_b[:, None]
            ).astype(BF),
            "wqkvT": np.ascontiguousarray(wqkvT),
            "woT": np.ascontiguousarray(woT),
            "gwT": np.ascontiguousarray(gwT),
            "lnq_g": lnq_g,
            "lnq_b": lnq_b2,
            "lnk_g": lnk_g,
            "lnf": lnf,
        }
        if flags[0]:
            m["bqkv"] = np.ascontiguousarray(bqkv).astype(BF)
        if flags[1]:
            m["bo_a"] = np.ascontiguousarray(bo_a).astype(BF)
        if flags[2]:
            m["gb"] = np.ascontiguousarray(gb_[None]).astype(BF)
        in_maps.append(m)
    return in_maps, flags


def kernel(trace=False, **inputs):
    from concourse.bass_utils import run_bass_kernel_spmd

    in_maps, flags = _prepare_in_maps(inputs)
    key = ("nc",) + flags
    if key not in _CACHE:
        _CACHE[key] = _build(*flags)
    nc = _CACHE[key]
    _CACHE["nc"] = nc
    res = run_bass_kernel_spmd(
        nc, in_maps, core_ids=list(range(CORES)), trace=trace
    )
    out = np.empty((B, N, D), np.float32)
    for c in range(CORES):
        b, r0 = c // GROUP, (c % GROUP) * S
        out[b, r0:r0 + S] = res.results[c]["out_rows"]
    _CACHE["last_result"] = res
    return out


# revision 64
# speedup vs baseline: 1.0211x; 1.0211x over previous
"""Trainium2 Bass kernel for nn_CustomCrossModalAttention (B=2, N=2048, D=768, H=12).

Sharding (8 cores, ZERO collectives):
  - core c owns batch b = c//4 and query rows [512*(c%4), 512*(c%4)+512).
  - k' and v are computed REDUNDANTLY for all 2048 keys of the core's batch
    (the cost-model prices AllGather at 15us + out_bytes/40GB/s, so the two
    baseline gathers cost 267us -- far more than the +46us of replicated PE
    matmul work).

Structure (engine-balance driven):
  phase1: q proj -> k' proj (transposed copy-out with pos fold) -> v chunks
          0..11. All LN rstds here use ACT Sqrt + DVE reciprocal -- they all
          complete BEFORE the first exp so the ACT table never thrashes.
  attention: heads stream with 3-chunk score groups -> exp (ACT-bound).
          v chunks 12..15 are interleaved into heads 0..1 with a DVE-only
          Newton rsqrt (no ACT table) + ACT Identity apply (Identity lives
          in every table). attn@v is re-oriented to [queries, dims] (full
          M=128) and lags the exp stream by 2 heads (at pool bufs=3).
  phase3: out proj, gate (sigmoid = 1/(1+exp(-x)) built from the already
          loaded exp table), fuse, final LN (single Sqrt table load).

Algebra (exact): scores*scale + q@pos == scale * (q @ (k + pos/scale)^T);
LN_v gain/bias folded into wo / bo on the host; softmax denominator via a
ones column appended to v.

Hardware constraints honored: GPSIMD never touches PSUM; all matmuls and
PE transposes in bf16 (1 cyc/row); psum pools sized to the 8x2KB banks.
"""

import numpy as np
import ml_dtypes

B, N, D = 2, 2048, 768
H, DH = 12, 64
P = 128
CORES, GROUP = 8, 4
S = 512            # query rows per core
NCH = S // P       # 4 own row chunks
MCH = N // P       # 16 key row chunks
G6 = D // P        # 6
SCALE = DH ** -0.5
EPS = 1e-5

BF = ml_dtypes.bfloat16

_CACHE = {}


def _build(has_bqkv, has_bo, has_gb):
    from contextlib import ExitStack

    import concourse.bacc as bacc
    import concourse.mybir as mybir
    import concourse.tile as tile
    from concourse.masks import make_identity

    f32 = mybir.dt.float32
    bf16 = mybir.dt.bfloat16
    ALU = mybir.AluOpType
    ACTF = mybir.ActivationFunctionType

    nc = bacc.Bacc("TRN2", target_bir_lowering=False, num_devices=CORES)

    def din(name, shape, dt=bf16):
        return nc.dram_tensor(name, shape, dt, kind="ExternalInput")

    xqT = din("xqT", [D, S])            # own infrared rows, transposed
    xvTf = din("xvTf", [D, N])          # FULL batch visible rows, transposed
    visT_own = din("visT_own", [D, S])  # own visible rows, transposed (gate)
    vis_nat = din("vis_nat", [S, D], f32)
    posTb = din("posTb", [D, N])        # (pos/scale + lnk_b), transposed
    wqkvT = din("wqkvT", [D, 3 * (D + 1)])
    woT = din("woT", [D, D])            # (wo * lnv_w).T
    gwT = din("gwT", [2 * D, D])
    lnq_g = din("lnq_g", [P, G6], f32)
    lnq_b = din("lnq_b", [P, G6], f32)
    lnk_g = din("lnk_g", [P, G6], f32)
    lnf = din("lnf", [2, D])
    bqkv = din("bqkv", [1, 3 * (D + 1)]) if has_bqkv else None
    bo_a = din("bo_a", [1, D]) if has_bo else None
    gb = din("gb", [1, D]) if has_gb else None
    out_rows = nc.dram_tensor("out_rows", [S, D], f32, kind="ExternalOutput")

    VPH1 = 12        # v chunks computed in phase 1 (ACT sqrt era)

    with tile.TileContext(nc) as tc, ExitStack() as ctx:
        const = ctx.enter_context(tc.tile_pool(name="const", bufs=1))

        ident = const.tile([P, P], bf16)
        make_identity(nc, ident)
        eps_t = const.tile([P, 1], f32)
        nc.vector.memset(eps_t, EPS)
        one_t = const.tile([P, 1], f32)
        nc.vector.memset(one_t, 1.0)
        ones_bf = const.tile([1, P], bf16)
        nc.vector.memset(ones_bf, 1.0)

        lnq_g_sb = const.tile([P, G6], f32)
        nc.scalar.dma_start(out=lnq_g_sb, in_=lnq_g.ap())
        lnq_b_sb = const.tile([P, G6], f32)
        nc.scalar.dma_start(out=lnq_b_sb, in_=lnq_b.ap())
        lnk_g_sb = const.tile([P, G6], f32)
        nc.scalar.dma_start(out=lnk_g_sb, in_=lnk_g.ap())
        woT_sb = const.tile([P, G6, D], bf16)
        nc.scalar.dma_start(out=woT_sb,
                            in_=woT.rearrange("(s p) o -> p s o", p=P))

        qT_sb = const.tile([P, G6, S], bf16)
        kT_sb = const.tile([P, G6, N], bf16)
        outT_sb = const.tile([P, G6, S], bf16)
        if has_bqkv:
            bqkv_sb = const.tile([1, 3 * (D + 1)], bf16)
            nc.scalar.dma_start(out=bqkv_sb, in_=bqkv.ap())

        HALves = [(0, 512), (512, D + 1)]
        HAL768 = [(0, 512), (512, D)]

        def proj_chunk(py, lhsT_sb, w_sb, w_off, c):
            for o0, o1 in HALves:
                for s in range(G6):
                    nc.tensor.matmul(
                        py[:, o0:o1],
                        lhsT_sb[:, s, c * P:(c + 1) * P],
                        w_sb[:, s, o0:o1],
                        start=(s == 0), stop=(not has_bqkv and s == G6 - 1),
                    )
                if has_bqkv:
                    nc.tensor.matmul(
                        py[:, o0:o1], ones_bf,
                        bqkv_sb[:, w_off + o0:w_off + o1],
                        start=False, stop=True,
                    )

        def stats(py, pool):
            st = pool.tile([P, 2, 6], f32, tag="st")
            for i in range(2):
                nc.vector.bn_stats(out=st[:, i], in_=py[:, i * 384:(i + 1) * 384])
            mv = pool.tile([P, 2], f32, tag="mv")
            nc.vector.bn_aggr(out=mv, in_=st)
            return mv

        def rstd_of(mv, pool, tag="rs"):
            """rstd = 1/sqrt(var+eps): ACT Sqrt then DVE reciprocal."""
            rstd = pool.tile([P, 1], f32, tag=tag + "r")
            nc.scalar.activation(out=rstd, in_=mv[:, 1:2], func=ACTF.Sqrt,
                                 bias=eps_t, scale=1.0)
            nc.vector.reciprocal(out=rstd, in_=rstd)
            return rstd

        def negmr_of(mv, rstd, pool, tag="nm"):
            negmr = pool.tile([P, 1], f32, tag=tag)
            nc.vector.tensor_scalar(
                out=negmr, in0=mv[:, 0:1], scalar1=rstd, scalar2=-1.0,
                op0=ALU.mult, op1=ALU.mult,
            )
            return negmr

        mid = ctx.enter_context(tc.tile_pool(name="mid", bufs=1))
        vaug_sb = mid.tile([P, MCH, H, DH + 1], bf16)
        nc.vector.memset(vaug_sb[:, :, :, DH:DH + 1], 1.0)
        onat_sb = mid.tile([P, NCH, H, DH], bf16)

        rinp = ctx.enter_context(tc.tile_pool(name="rin", bufs=4))
        w2 = ctx.enter_context(tc.tile_pool(name="w2", bufs=4))

        GROUPS6 = [(15, 1), (0, 3), (3, 3), (6, 3), (9, 3), (12, 3)]

        def scores_group(ps_pool, at_h, h, mc0, w):
            p0 = DH * (h % 2)
            grp = h // 2
            ps = ps_pool.tile([P, 3, S], f32, tag="ps3")
            for j in range(w):
                mc = mc0 + j
                nc.tensor.matmul(
                    ps[:, j],
                    kT_sb[p0:p0 + DH, grp, mc * P:(mc + 1) * P],
                    qT_sb[p0:p0 + DH, grp, :],
                    start=True, stop=True,
                )
            nc.scalar.activation(
                out=at_h[:, mc0:mc0 + w, :], in_=ps[:, :w],
                func=ACTF.Exp, scale=SCALE,
            )

        def av_head(ps_o, at_h, h):
            po = ps_o.tile([P, NCH, DH + 1], f32, tag="po")
            for qc in range(NCH):
                for mc in range(MCH):
                    nc.tensor.matmul(
                        po[:, qc],
                        at_h[:, mc, qc * P:(qc + 1) * P],
                        vaug_sb[:, mc, h, :],
                        start=(mc == 0), stop=(mc == MCH - 1),
                    )
                rinv = rinp.tile([P, 1], f32, tag="rin")
                nc.vector.reciprocal(out=rinv, in_=po[:, qc, DH:DH + 1])
                nc.vector.tensor_scalar_mul(
                    out=onat_sb[:, qc, h], in0=po[:, qc, 0:DH], scalar1=rinv,
                )

        def newton_rstd(mv):
            """rsqrt(var+eps) purely on DVE: r0 = 1/(var+eps), then two
            Newton steps r <- r*(1.5 - 0.5*x*r^2). Converges fast since the
            projection row variance concentrates near 1."""
            x = w2.tile([P, 1], f32, tag="nx")
            nc.vector.tensor_scalar_add(out=x, in0=mv[:, 1:2], scalar1=EPS)
            r = w2.tile([P, 1], f32, tag="nr")
            nc.vector.reciprocal(out=r, in_=x)
            a = w2.tile([P, 1], f32, tag="na")
            for _ in range(2):
                nc.vector.tensor_tensor(out=a, in0=r, in1=r, op=ALU.mult)
                nc.vector.tensor_tensor(out=a, in0=a, in1=x, op=ALU.mult)
                nc.vector.tensor_scalar(
                    out=a, in0=a, scalar1=-0.5, scalar2=1.5,
                    op0=ALU.mult, op1=ALU.add,
                )
                nc.vector.tensor_tensor(out=r, in0=r, in1=a, op=ALU.mult)
            return r

        at_tiles = {}
        atp_cm = tc.tile_pool(name="atp", bufs=3)
        atp = atp_cm.__enter__()
        with tc.tile_pool(name="xvw", bufs=1) as xvw:
            xvT_sb = xvw.tile([P, G6, N], bf16)
            wv_sb = xvw.tile([P, G6, D + 1], bf16)

            # ---------------- phase 1: q, k', v[0:12] ----------------
            with (
                tc.tile_pool(name="ph1", bufs=1) as ph1,
                tc.tile_pool(name="knp", bufs=3) as knp,
                tc.tile_pool(name="post", bufs=2) as postp,
                tc.tile_pool(name="stat", bufs=4) as statp,
                tc.tile_pool(name="ps_p", bufs=3, space="PSUM") as ps_p,
                tc.tile_pool(name="ps_t", bufs=1, space="PSUM") as ps_t,
            ):
                xqT_sb = ph1.tile([P, G6, S], bf16)
                nc.sync.dma_start(out=xqT_sb,
                                  in_=xqT.rearrange("(s p) n -> p s n", p=P))
                nc.gpsimd.dma_start(out=xvT_sb,
                                    in_=xvTf.rearrange("(s p) n -> p s n", p=P))
                wq_sb = ph1.tile([P, G6, D + 1], bf16)
                for h0, h1 in HALves:
                    nc.sync.dma_start(
                        out=wq_sb[:, :, h0:h1],
                        in_=wqkvT.rearrange("(s p) o -> p s o", p=P)[:, :, h0:h1],
                    )
                wk_sb = ph1.tile([P, G6, D + 1], bf16)
                nc.sync.dma_start(
                    out=wk_sb,
                    in_=wqkvT.rearrange("(s p) o -> p s o", p=P)[
                        :, :, D + 1:2 * (D + 1)],
                )
                nc.sync.dma_start(
                    out=wv_sb,
                    in_=wqkvT.rearrange("(s p) o -> p s o", p=P)[
                        :, :, 2 * (D + 1):3 * (D + 1)],
                )
                qnat = ph1.tile([P, NCH, D], bf16)

                for c in range(NCH):
                    py = ps_p.tile([P, D + 1], f32, tag="py")
                    proj_chunk(py, xqT_sb, wq_sb, 0, c)
                    mv = stats(py, statp)
                    rstd = rstd_of(mv, statp, "qr")
                    negmr = negmr_of(mv, rstd, statp, "qn")
                    nc.scalar.activation(
                        out=qnat[:, c], in_=py[:, 0:D], func=ACTF.Identity,
                        bias=negmr, scale=rstd,
                    )

                scrp = ph1.tile([P, 2, D], bf16)

                def kchunk(c, pend):
                    py = ps_p.tile([P, D + 1], f32, tag="py")
                    proj_chunk(py, xvT_sb, wk_sb, D + 1, c)
                    # stats on ACT: sum(y^2) via Square+accumulate; mean came
                    # free out of the matmul's appended wsum/768 column.
                    sumsq = statp.tile([P, 1], f32, tag="ksq")
                    nc.scalar.activation(
                        out=scrp[:, c % 2], in_=py[:, 0:D], func=ACTF.Square,
                        accum_out=sumsq,
                    )
                    mean = statp.tile([P, 1], f32, tag="kmn")
                    nc.vector.tensor_scalar_mul(out=mean, in0=py[:, D:D + 1],
                                                scalar1=1.0)
                    var = statp.tile([P, 1], f32, tag="kvr")
                    nc.vector.tensor_scalar(
                        out=var, in0=mean, scalar1=mean, scalar2=-1.0,
                        op0=ALU.mult, op1=ALU.mult,
                    )
                    nc.vector.tensor_scalar(
                        out=var, in0=sumsq, scalar1=1.0 / D, scalar2=var,
                        op0=ALU.mult, op1=ALU.add,
                    )
                    mv = statp.tile([P, 2], f32, tag="kmv")
                    nc.vector.tensor_copy(out=mv[:, 0:1], in_=mean)
                    nc.vector.tensor_copy(out=mv[:, 1:2], in_=var)
                    rstd = rstd_of(mv, statp, "kr")
                    negmr = negmr_of(mv, rstd, statp, "kn")
                    pair = c // 2
                    if c % 2 == 0:
                        pend["t"] = knp.tile([P, 2, D], bf16, tag="knat",
                                             name=f"knat{c}")
                    nc.scalar.activation(
                        out=pend["t"][:, c % 2], in_=py[:, 0:D],
                        func=ACTF.Identity, bias=negmr, scale=rstd,
                    )
                    pend[pair] = pend["t"]

                def ktranspose(pair, pend):
                    knat2 = pend.pop(pair)
                    pos_sb = postp.tile([P, G6, 2 * P], bf16, tag="pos")
                    nc.sync.dma_start(
                        out=pos_sb,
                        in_=posTb.rearrange("(s p) n -> p s n", p=P)[
                            :, :, pair * 2 * P:(pair + 1) * 2 * P],
                    )
                    pt = ps_t.tile([P, G6, 2 * P], bf16, tag="pt")
                    for i in range(2):
                        for s in range(G6):
                            nc.tensor.transpose(
                                pt[:, s, i * P:(i + 1) * P],
                                knat2[:, i, s * P:(s + 1) * P], ident,
                            )
                    for s in range(G6):
                        nc.vector.scalar_tensor_tensor(
                            out=kT_sb[:, s, pair * 2 * P:(pair + 1) * 2 * P],
                            in0=pt[:, s], scalar=lnk_g_sb[:, s:s + 1],
                            in1=pos_sb[:, s], op0=ALU.mult, op1=ALU.add,
                        )

                pend = {}
                for c in range(MCH):
                    kchunk(c, pend)
                    if c == 1:
                        for g in range(2):
                            ptq = ps_t.tile([P, G6, 2 * P], bf16, tag="pt")
                            for i in range(2):
                                qc = g * 2 + i
                                for s in range(G6):
                                    nc.tensor.transpose(
                                        ptq[:, s, i * P:(i + 1) * P],
                                        qnat[:, qc, s * P:(s + 1) * P], ident,
                                    )
                            for s in range(G6):
                                nc.vector.scalar_tensor_tensor(
                                    out=qT_sb[:, s, g * 2 * P:(g + 1) * 2 * P],
                                    in0=ptq[:, s], scalar=lnq_g_sb[:, s:s + 1],
                                    in1=lnq_b_sb[:, s:s + 1].to_broadcast(
                                        [P, 2 * P]),
                                    op0=ALU.mult, op1=ALU.add,
                                )
                    if c >= 3 and c % 2 == 1:
                        ktranspose((c - 1) // 2 - 1, pend)
                ktranspose(MCH // 2 - 1, pend)

                # v chunks 0..11: rstd via ACT sqrt (still before any exp)
                for c in range(VPH1):
                    pv = ps_p.tile([P, D + 1], f32, tag="py")
                    proj_chunk(pv, xvT_sb, wv_sb, 2 * (D + 1), c)
                    mv = stats(pv, statp)
                    rstd = rstd_of(mv, statp, "vr")
                    negmr = negmr_of(mv, rstd, statp, "vn")
                    nc.scalar.activation(
                        out=vaug_sb[:, c, :, 0:DH],
                        in_=pv[:, 0:D].rearrange("p (h d) -> p h d", h=H),
                        func=ACTF.Identity, bias=negmr, scale=rstd,
                    )

            # ---------------- attention part A: heads 0..1 + v[12:16] ------
            with (
                tc.tile_pool(name="ps_sA", bufs=2, space="PSUM") as ps_sA,
                tc.tile_pool(name="ps_v", bufs=1, space="PSUM") as ps_v,
            ):
                vq = list(range(VPH1, MCH))
                for h in range(2):
                    at_h = atp.tile([P, MCH, S], bf16, tag="at", name=f"at{h}")
                    at_tiles[h] = at_h
                    for gi, (mc0, w) in enumerate(GROUPS6):
                        if gi in (0, 3) and vq:
                            c = vq.pop(0)
                            pv = ps_v.tile([P, D + 1], f32, tag="pv")
                            proj_chunk(pv, xvT_sb, wv_sb, 2 * (D + 1), c)
                            mv = stats(pv, w2)
                            r = newton_rstd(mv)
                            negmr = negmr_of(mv, r, w2, "wn")
                            nc.scalar.activation(
                                out=vaug_sb[:, c, :, 0:DH],
                                in_=pv[:, 0:D].rearrange("p (h d) -> p h d", h=H),
                                func=ACTF.Identity, bias=negmr, scale=r,
                            )
                        scores_group(ps_sA, at_h, h, mc0, w)

        # ---------------- attention part B: heads 2..11 + lagged av --------
        with (
            tc.tile_pool(name="ps_o", bufs=2, space="PSUM") as ps_o,
            tc.tile_pool(name="ps_sB", bufs=2, space="PSUM") as ps_sB,
        ):
            for h in range(2, H):
                at_h = atp.tile([P, MCH, S], bf16, tag="at", name=f"at{h}")
                at_tiles[h] = at_h
                if h == H - 1:
                    # last head: keep its score stream dense so the final
                    # exps (and the whole tail) start as early as possible
                    av_head(ps_o, at_tiles.pop(h - 2), h - 2)
                for gi, (mc0, w) in enumerate(GROUPS6):
                    scores_group(ps_sB, at_h, h, mc0, w)
                    if gi == 0 and h < H - 1:
                        # av for h-2 after this head's first exp is fed
                        av_head(ps_o, at_tiles.pop(h - 2), h - 2)
            av_head(ps_o, at_tiles.pop(H - 2), H - 2)
            av_head(ps_o, at_tiles.pop(H - 1), H - 1)
        atp_cm.__exit__(None, None, None)

        # transpose attention output for the output projection
        with tc.tile_pool(name="ps_t2", bufs=1, space="PSUM") as ps_t2:
            for g in range(2):
                pt = ps_t2.tile([P, G6, 2 * P], bf16, tag="pt2")
                for i in range(2):
                    qc = g * 2 + i
                    src = onat_sb[:, qc].rearrange("p h d -> p (h d)")
                    for s in range(G6):
                        nc.tensor.transpose(
                            pt[:, s, i * P:(i + 1) * P],
                            src[:, s * P:(s + 1) * P], ident,
                        )
                if g == 0:
                    nc.vector.tensor_copy(
                        out=outT_sb[:, :, g * 2 * P:(g + 1) * 2 * P],
                        in_=pt,
                    )
                else:
                    nc.scalar.copy(
                        out=outT_sb[:, :, g * 2 * P:(g + 1) * 2 * P],
                        in_=pt,
                    )

        # ---------------- phase 3: out proj, gate, fuse, final LN ----------
        with (
            tc.tile_pool(name="ph3", bufs=1) as ph3,
            tc.tile_pool(name="fw", bufs=2) as fw,
            tc.tile_pool(name="st3", bufs=4) as st3,
            tc.tile_pool(name="ps_z", bufs=2, space="PSUM") as ps_z,
            tc.tile_pool(name="ps_t3", bufs=1, space="PSUM") as ps_t3,
        ):
            ident_f32 = ph3.tile([P, P], f32)
            make_identity(nc, ident_f32)
            lnfw_sb = ph3.tile([1, D], bf16)
            nc.scalar.dma_start(out=lnfw_sb, in_=lnf.ap()[0:1, :])
            lnfb_sb = ph3.tile([1, D], bf16)
            nc.scalar.dma_start(out=lnfb_sb, in_=lnf.ap()[1:2, :])
            vis_sb = ph3.tile([P, NCH, D], f32)
            nc.scalar.dma_start(out=vis_sb,
                                in_=vis_nat.rearrange("(c p) o -> p c o", p=P))
            visT_sb = ph3.tile([P, G6, S], bf16)
            nc.scalar.dma_start(out=visT_sb,
                                in_=visT_own.rearrange("(s p) n -> p s n", p=P))
            gwv_sb = ph3.tile([P, G6, D], bf16)
            nc.sync.dma_start(
                out=gwv_sb,
                in_=gwT.rearrange("(s p) o -> p s o", p=P)[0:P, 0:G6, :],
            )
            gwz_sb = ph3.tile([P, G6, D], bf16)
            nc.sync.dma_start(
                out=gwz_sb,
                in_=gwT.rearrange("(g s p) o -> p (g s) o", p=P, g=2)[:, G6:, :],
            )
            if has_gb:
                gb_sb = ph3.tile([1, D], bf16)
                nc.sync.dma_start(out=gb_sb, in_=gb.ap())
            if has_bo:
                bo_sb = ph3.tile([1, D], bf16)
                nc.sync.dma_start(out=bo_sb, in_=bo_a.ap())

            z_sb = ph3.tile([P, NCH, D], f32)
            zT_sb = ph3.tile([P, G6, S], bf16)
            gsig = ph3.tile([P, NCH, D], f32)
            gbc = ph3.tile([P, D], f32)
            bbc = ph3.tile([P, D], f32)

            # broadcast final-LN gain/bias (cheap bf16; also warms PE)
            for dst, src_row in ((gbc, lnfw_sb), (bbc, lnfb_sb)):
                pb = ps_z.tile([P, D], f32, tag="pz")
                for o0, o1 in HAL768:
                    nc.tensor.matmul(
                        pb[:, o0:o1], ones_bf, src_row[:, o0:o1],
                        start=True, stop=True,
                    )
                nc.vector.tensor_copy(out=dst, in_=pb)

            # z = attout @ woT' (+bo)
            for c in range(NCH):
                pz = ps_z.tile([P, D], f32, tag="pz")
                for o0, o1 in HAL768:
                    for s in range(G6):
                        nc.tensor.matmul(
                            pz[:, o0:o1],
                            outT_sb[:, s, c * P:(c + 1) * P],
                            woT_sb[:, s, o0:o1],
                            start=(s == 0), stop=(not has_bo and s == G6 - 1),
                        )
                    if has_bo:
                        nc.tensor.matmul(
                            pz[:, o0:o1], ones_bf, bo_sb[:, o0:o1],
                            start=False, stop=True,
                        )
                nc.scalar.copy(out=z_sb[:, c], in_=pz)

            # zT for the gate matmul
            for g in range(2):
                pt = ps_t3.tile([P, G6, 2 * P], f32, tag="pt3")
                for i in range(2):
                    c = g * 2 + i
                    for s in range(G6):
                        nc.tensor.transpose(
                            pt[:, s, i * P:(i + 1) * P],
                            z_sb[:, c, s * P:(s + 1) * P], ident_f32,
                        )
                if g == 0:
                    nc.vector.tensor_copy(
                        out=zT_sb[:, :, g * 2 * P:(g + 1) * 2 * P], in_=pt,
                    )
                else:
                    nc.scalar.copy(
                        out=zT_sb[:, :, g * 2 * P:(g + 1) * 2 * P], in_=pt,
                    )

            # gate: sigmoid(x) = 1/(1+exp(-x)) -- exp stays in the exp table
            for c in range(NCH):
                pg = ps_z.tile([P, D], f32, tag="pz")
                for o0, o1 in HAL768:
                    for s in range(G6):
                        nc.tensor.matmul(
                            pg[:, o0:o1],
                            visT_sb[:, s, c * P:(c + 1) * P],
                            gwv_sb[:, s, o0:o1],
                            start=(s == 0), stop=False,
                        )
                    for s in range(G6):
                        nc.tensor.matmul(
                            pg[:, o0:o1],
                            zT_sb[:, s, c * P:(c + 1) * P],
                            gwz_sb[:, s, o0:o1],
                            start=False,
                            stop=(not has_gb and s == G6 - 1),
                        )
                    if has_gb:
                        nc.tensor.matmul(
                            pg[:, o0:o1], ones_bf, gb_sb[:, o0:o1],
                            start=False, stop=True,
                        )
                ge = fw.tile([P, D], f32, tag="ge", name=f"ge{c}")
                nc.scalar.activation(out=ge, in_=pg, func=ACTF.Exp, scale=-1.0)
                nc.scalar.activation(out=ge, in_=ge, func=ACTF.Identity,
                                     bias=one_t, scale=1.0)
                nc.vector.reciprocal(out=gsig[:, c], in_=ge)

            # fuse + final LN (stats first, one batched sqrt table load)
            fmv = ph3.tile([P, NCH, 2], f32)
            fus4 = gsig  # gsig is dead after the fuse multiply; reuse in place
            for c in range(NCH):
                dvz = fw.tile([P, D], f32, tag="dvz")
                nc.gpsimd.tensor_tensor(
                    out=dvz, in0=vis_sb[:, c], in1=z_sb[:, c], op=ALU.subtract,
                )
                nc.vector.tensor_tensor(out=fus4[:, c], in0=gsig[:, c],
                                        in1=dvz, op=ALU.mult)
                nc.vector.tensor_tensor(out=fus4[:, c], in0=fus4[:, c],
                                        in1=z_sb[:, c], op=ALU.add)
                st = st3.tile([P, 2, 6], f32, tag="st")
                for i in range(2):
                    nc.vector.bn_stats(out=st[:, i],
                                       in_=fus4[:, c, i * 384:(i + 1) * 384])
                nc.vector.bn_aggr(out=fmv[:, c], in_=st)
            frstd = ph3.tile([P, NCH], f32)
            nc.scalar.activation(out=frstd, in_=fmv[:, :, 1], func=ACTF.Sqrt,
                                 bias=eps_t, scale=1.0)
            nc.vector.reciprocal(out=frstd, in_=frstd)
            fnegmr = ph3.tile([P, NCH], f32)
            nc.vector.tensor_tensor(out=fnegmr, in0=fmv[:, :, 0], in1=frstd,
                                    op=ALU.mult)
            nc.vector.tensor_scalar_mul(out=fnegmr, in0=fnegmr, scalar1=-1.0)
            for c in range(NCH):
                tnorm = fw.tile([P, D], f32, tag="tn")
                nc.scalar.activation(
                    out=tnorm, in_=fus4[:, c], func=ACTF.Identity,
                    bias=fnegmr[:, c:c + 1], scale=frstd[:, c:c + 1],
                )
                nc.vector.tensor_tensor(out=tnorm, in0=tnorm, in1=gbc,
                                        op=ALU.mult)
                nc.vector.tensor_tensor(out=tnorm, in0=tnorm, in1=bbc,
                                        op=ALU.add)
                nc.sync.dma_start(
                    out=out_rows.rearrange("(c p) o -> p c o", p=P)[:, c],
                    in_=tnorm,
                )

    nc.compile()
    return nc


def _prepare_in_maps(inputs):
    f32 = np.float32
    vis = np.asarray(inputs["visible_features"], f32)
    inf = np.asarray(inputs["infrared_features"], f32)
    wq = np.asarray(inputs["wq"], f32)
    bq = np.asarray(inputs["bq"], f32)
    lnq_w = np.asarray(inputs["lnq_w"], f32)
    lnq_b = np.asarray(inputs["lnq_b"], f32)
    wk = np.asarray(inputs["wk"], f32)
    bk = np.asarray(inputs["bk"], f32)
    lnk_w = np.asarray(inputs["lnk_w"], f32)
    lnk_b = np.asarray(inputs["lnk_b"], f32)
    wv = np.asarray(inputs["wv"], f32)
    bv = np.asarray(inputs["bv"], f32)
    lnv_w = np.asarray(inputs["lnv_w"], f32)
    lnv_b = np.asarray(inputs["lnv_b"], f32)
    pos = np.asarray(inputs["pos_emb"], f32)[:N]
    wo = np.asarray(inputs["wo"], f32)
    bo = np.asarray(inputs["bo"], f32)
    gw = np.asarray(inputs["gate_w"], f32)
    gb_ = np.asarray(inputs["gate_b"], f32)
    ln_w = np.asarray(inputs["ln_w"], f32)
    ln_b = np.asarray(inputs["ln_b"], f32)

    blocks = []
    for w in (wq, wk, wv):
        wt = w.T.astype(np.float32)
        blocks.append(np.concatenate([wt, wt.mean(axis=1, keepdims=True)],
                                     axis=1))
    wqkvT = np.concatenate(blocks, axis=1).astype(BF)
    bqkv = np.concatenate(
        [np.concatenate([b, [b.mean()]]) for b in (bq, bk, bv)])[None]
    woT = ((wo * lnv_w[None, :]).T).astype(BF)
    bo_a = (bo + wo @ lnv_b)[None]
    gwT = gw.T.astype(BF)
    lnq_g = np.ascontiguousarray(lnq_w.reshape(G6, P).T)
    lnq_b2 = np.ascontiguousarray(lnq_b.reshape(G6, P).T)
    lnk_g = np.ascontiguousarray(lnk_w.reshape(G6, P).T)
    lnf = np.stack([ln_w, ln_b]).astype(BF)
    flags = (
        bool(np.any(bqkv)), bool(np.any(bo_a)), bool(np.any(gb_)),
    )

    in_maps = []
    for c in range(CORES):
        b, r0 = c // GROUP, (c % GROUP) * S
        m = {
            "xqT": np.ascontiguousarray(inf[b, r0:r0 + S].T).astype(BF),
            "xvTf": np.ascontiguousarray(vis[b].T).astype(BF),
            "visT_own": np.ascontiguousarray(vis[b, r0:r0 + S].T).astype(BF),
            "vis_nat": np.ascontiguousarray(vis[b, r0:r0 + S]),
            "posTb": np.ascontiguousarray(
                pos.T / SCALE + lnk_b[:, None]
            ).astype(BF),
            "wqkvT": np.ascontiguousarray(wqkvT),
            "woT": np.ascontiguousarray(woT),
            "gwT": np.ascontiguousarray(gwT),
            "lnq_g": lnq_g,
            "lnq_b": lnq_b2,
            "lnk_g": lnk_g,
            "lnf": lnf,
        }
        if flags[0]:
            m["bqkv"] = np.ascontiguousarray(bqkv).astype(BF)
        if flags[1]:
            m["bo_a"] = np.ascontiguousarray(bo_a).astype(BF)
        if flags[2]:
            m["gb"] = np.ascontiguousarray(gb_[None]).astype(BF)
        in_maps.append(m)
    return in_maps, flags


def kernel(trace=False, **inputs):
    from concourse.bass_utils import run_bass_kernel_spmd

    in_maps, flags = _prepare_in_maps(inputs)
    key = ("nc",) + flags
    if key not in _CACHE:
        _CACHE[key] = _build(*flags)
    nc = _CACHE[key]
    _CACHE["nc"] = nc
    res = run_bass_kernel_spmd(
        nc, in_maps, core_ids=list(range(CORES)), trace=trace
    )
    out = np.empty((B, N, D), np.float32)
    for c in range(CORES):
        b, r0 = c // GROUP, (c % GROUP) * S
        out[b, r0:r0 + S] = res.results[c]["out_rows"]
    _CACHE["last_result"] = res
    return out
